# revision 7
# baseline (speedup 1.0000x reference)
"""Trainium2 Bass kernel for the BaseMemory coref scoring module.

Computes, for full inputs (M=65536 memory slots, D=768, E=20, H=64):
    score = relu(pair @ W1 + b1) @ W2 + b2, masked with ent_counter>0,
    where pair = [mem, ment, mem*ment, dist_emb, cnt_emb].

Sharding: data-parallel over the cluster dimension M across 8 NeuronCores.

Key algebraic folds (host side, O(D*H + M*D) work, no M*D*H matmul):
  - mem@W1_mem + (mem*ment)@W1_had = mem @ W  with W = W1_mem + diag(ment)@W1_had
  - the whole per-row additive term t_m = b1 + ment@W1_ment
      + dist_table[bd_m]@W1_dist + counter_table[bc_m]@W1_cnt  (only 100
    distinct values over the two 10-way buckets) is folded INTO the data
    stream:  x'_m = mem_m + Wp t_m  with  Wp = W (W^T W)^{-1}, so that
    W^T x'_m = W^T mem_m + t_m exactly.  The device then only computes
    relu(W^T x') @ W2 -- two matmuls, nothing else.
  - masking (+b2, -10000 on empty slots, trailing 0) is applied during the
    host-side gather, as is the trailing new-cluster slot.
  - x' and W are cast to bf16: halves HBM traffic (the kernel is
    memory-bound) at ~4e-3 worst-case relative error, well inside 2e-2.
"""

import os
import numpy as np
from ml_dtypes import bfloat16

# The bass kernel executes through the axon PJRT backend; make sure jax can
# see it even if the caller pinned JAX_PLATFORMS (e.g. to "cpu").
_jp = os.environ.get("JAX_PLATFORMS")
if _jp is not None and _jp != "" and "axon" not in _jp:
    os.environ["JAX_PLATFORMS"] = "axon," + _jp

M, D, E, H = 65536, 768, 20, 64
N_CORES = 8
MS = M // N_CORES          # rows per core = 8192
GROUP = 512                # rows per PE matmul group
N_GROUPS = MS // GROUP     # 16
KCH = D // 128             # 6 contraction chunks
SG = 4                     # groups per output DMA
N_SG = N_GROUPS // SG      # 4
# DMA pieces (in groups). The stream (not the PE) is the roofline, so the
# first piece is large: PE starts late enough that it then never stalls on
# piece boundaries (each stall costs a pipeline refill + p-state dip), and
# finishes right behind the last (small) piece.
PIECES = (3, 2, 2, 2, 2, 2, 2, 1)
assert sum(PIECES) == N_GROUPS

_CACHE = {}


def _build():
    """Build + compile the 8-core SPMD bass program once per process."""
    if "nc" in _CACHE:
        return _CACHE["nc"]

    import concourse.bass as bass
    import concourse.mybir as mybir
    import concourse.tile as tile
    from concourse import bacc

    F32 = mybir.dt.float32
    F32R = mybir.dt.float32r
    BF16 = mybir.dt.bfloat16

    nc = bacc.Bacc("TRN2", target_bir_lowering=False, debug=False,
                   enable_asserts=False, num_devices=N_CORES)

    # xt[p, g, k, c] = x'[k*128 + p, g*512 + c]  (bf16, DMA-friendly layout:
    # each partition's slice for a run of groups is contiguous)
    xt_d = nc.dram_tensor("xt", [128, N_GROUPS, KCH, GROUP], BF16,
                          kind="ExternalInput").ap()
    # w1 chunks and the W2 column packed into one bf16 const buffer
    w1_d = nc.dram_tensor("w1", [128, KCH * H + 1], BF16,
                          kind="ExternalInput").ap()
    out_d = nc.dram_tensor("out", [MS], F32, kind="ExternalOutput").ap()
    out_r = out_d.rearrange("(s c) -> s c", s=N_SG)  # [4, 2048]

    relu = mybir.ActivationFunctionType.Relu

    with tile.TileContext(nc) as tc:
        with (
            tc.tile_pool(name="consts", bufs=1) as cpool,
            tc.tile_pool(name="xin", bufs=1) as px,
            tc.tile_pool(name="ht", bufs=6) as pht,
            tc.tile_pool(name="osb", bufs=2) as posb,
            tc.tile_pool(name="psz", bufs=4, space="PSUM") as psz,
            tc.tile_pool(name="pss", bufs=2, space="PSUM") as pss,
        ):
            # consts first on the sync queue: tiny (99KB), and the xt
            # stream behind them is what gates everything anyway
            w1f = cpool.tile([128, KCH * H + 1], BF16, tag="w1t")
            nc.sync.dma_start(w1f[:], w1_d[:])
            wsc = w1f[0:H, KCH * H:KCH * H + 1]

            def load_piece(i, g0, ng):
                xk = px.tile([128, ng, KCH, GROUP], BF16, tag=f"xin{i}")
                nc.sync.dma_start(xk[:], xt_d[:, g0:g0 + ng, :, :])
                return xk

            tiles = []
            g0 = 0
            for i, ng in enumerate(PIECES):
                tiles.append((g0, ng, load_piece(i, g0, ng)))
                g0 += ng

            osb_tiles = {}
            pending = None

            def emit_score(g, ht):
                sc = pss.tile([1, GROUP], F32, tag="pss")
                nc.tensor.matmul(sc[:], wsc, ht[:], start=True, stop=True)
                sq = g // SG
                if g % SG == 0:
                    osb_t = posb.tile([1, SG * GROUP], F32, tag="osb")
                    osb_tiles[sq] = osb_t
                orow = osb_tiles[sq][0:1, GROUP * (g % SG):GROUP * (g % SG + 1)]
                # odd groups on vector: the scalar engine is busy with the
                # relu of the same group (matters for the last group's tail)
                if g % 2 == 0:
                    nc.scalar.copy(orow, sc[:])
                else:
                    nc.vector.tensor_copy(orow, sc[:])
                if g % SG == SG - 1:
                    nc.gpsimd.dma_start(out_r[sq:sq + 1, :],
                                        osb_tiles.pop(sq)[:])

            for g0, ng, xk in tiles:
                for gi in range(ng):
                    g = g0 + gi
                    if pending is not None:
                        emit_score(*pending)

                    zt = psz.tile([H, GROUP], F32, tag="psz")
                    for k in range(KCH):
                        nc.tensor.matmul(zt[:], w1f[:, H * k:H * (k + 1)],
                                         xk[:, gi, k, :],
                                         start=(k == 0), stop=(k == KCH - 1))

                    ht = pht.tile([H, GROUP], BF16, tag="ht")
                    nc.scalar.activation(ht[:], zt[:], relu)
                    pending = (g, ht)
            emit_score(*pending)

    nc.compile()
    _CACHE["nc"] = nc
    return nc


_BOUNDS = np.array([1, 2, 3, 4, 5, 8, 16, 32, 64], np.int64)


def _bucket(c):
    """Identity buckets for c<=4, log2 buckets above, clamped to [0, 9].
    Integer-exact equivalent of the reference's float bucketing."""
    return np.searchsorted(_BOUNDS, np.asarray(c, np.int64), side="right")


def _prepare_maps(ment_emb, mem_vectors, dist_table, counter_table,
                  W1, b1, W2, b2, ent_counter, last_mention_start, ment_start):
    f64 = np.float64
    ment = np.asarray(ment_emb, f64)
    W1 = np.asarray(W1, f64)

    W1m, W1r, W1h = W1[0:D], W1[D:2 * D], W1[2 * D:3 * D]
    W1d, W1c = W1[3 * D:3 * D + E], W1[3 * D + E:3 * D + 2 * E]

    W = W1m + ment[:, None] * W1h                       # [768, 64]
    bias = np.asarray(b1, f64) + ment @ W1r             # [64]
    Td = np.asarray(dist_table, f64) @ W1d + bias       # [10, 64]
    Tc = np.asarray(counter_table, f64) @ W1c           # [10, 64]
    # Wp = W (W^T W)^{-1}; W^T (x + Wp t) = W^T x + t exactly
    Wp = np.linalg.solve(W.T @ W, W.T).T                # [768, 64]
    T_all = (Td[:, None, :] + Tc[None, :, :]).reshape(100, H)
    Delta = (T_all @ Wp.T).astype(np.float32)           # [100, 768]

    cnt = np.asarray(ent_counter, np.int64)
    dist = int(np.asarray(ment_start)) - np.asarray(last_mention_start,
                                                    np.int64)
    idx = _bucket(dist) * 10 + _bucket(cnt)             # [M]

    mem = np.asarray(mem_vectors, np.float32)
    xp = mem + Delta[idx]                               # [M, 768] f32
    w1b = np.zeros((128, KCH * H + 1), bfloat16)
    w1b[:, :KCH * H] = (W.astype(np.float32).astype(bfloat16)
                        .reshape(KCH, 128, H).transpose(1, 0, 2)
                        .reshape(128, KCH * H))
    w1b[:H, KCH * H] = np.asarray(W2, np.float32).astype(bfloat16).reshape(H)

    in_maps = []
    for c in range(N_CORES):
        sl = slice(c * MS, (c + 1) * MS)
        a = xp[sl].T.reshape(KCH, 128, N_GROUPS, GROUP)
        xt = np.ascontiguousarray(a.transpose(1, 2, 0, 3)).astype(bfloat16)
        in_maps.append(dict(xt=xt, w1=w1b))

    _CACHE["mask"] = cnt == 0
    _CACHE["b2"] = float(np.asarray(b2, np.float64).reshape(-1)[0])
    return in_maps


def _postprocess(results):
    out = np.empty(M + 1, np.float32)
    for c in range(N_CORES):
        out[c * MS:(c + 1) * MS] = results[c]["out"]
    out[:M] += _CACHE["b2"]
    out[:M][_CACHE["mask"]] = -10000.0
    out[M] = 0.0
    return out


def run_spmd(in_maps, trace=False):
    from concourse.bass_utils import run_bass_kernel_spmd
    nc = _build()
    return run_bass_kernel_spmd(nc, in_maps, list(range(N_CORES)), trace=trace)


def kernel(**inputs):
    in_maps = _prepare_maps(**inputs)
    res = run_spmd(in_maps, trace=False)
    return _postprocess(res.results)
